# revision 11
# baseline (speedup 1.0000x reference)
"""AngleLoss distributed Trainium2 kernel, v2.

mean(arccos(dot(o,t)/(|o||t|))) over 2,097,152 rows of 3-vectors,
data-parallel over 8 NeuronCores. Host pre-rounds inputs to bf16
(halves HBM traffic; rel-err budget 2e-2 is ~100x above the cost).

Math per row (one LUT chain, no explicit cos):
    dot = sum o*t ; prod = (sum o^2)(sum t^2)
    h   = dot * absrsqrt(|prod - dot^2|)   # = cot(theta)
    theta = pi/2 - arctan(h)               # arctan covers +-inf -> +-pi/2
Eliminates v1's r1/c/c2/nump stages (saves ~4C VE elems + 1 ScalarE LUT).
Device accumulates sum(arctan(h)); host computes pi/2 - total/R.

Per-tile engine split (tunable): squares of the 6 planes are divided
between ScalarE (Square activation), GpSimd (tensor_tensor), and VE;
pd = prod - dot^2 optionally on GpSimd. VE does m/dot/pairs/prod/h.

Layout: per core, tile-major planar: tile i = [128 part x 6*F_i bf16]
with each partition's 6*F_i values contiguous (planes o0,o1,o2,t0,t1,t2).
"""

import sys

import numpy as np

if "/opt/trn_rl_repo" not in sys.path:
    sys.path.insert(0, "/opt/trn_rl_repo")

N_CORES = 8
R_TOTAL = 256 * 8192  # 2097152 rows
PER_CORE = R_TOTAL // N_CORES  # 262144
P = 128
FREE = PER_CORE // P  # 2048

import os as _os

_ts = _os.environ.get("ANGLE_TILE_SIZES")
TILE_SIZES = tuple(int(v) for v in _ts.split(",")) if _ts else (128, 704, 832, 384)
N_INBUF = len(TILE_SIZES) + 1  # all tiles resident + 1
SQ_SC = int(_os.environ.get("ANGLE_SQ_SC", "4"))  # planes squared on ScalarE
SQ_GP = int(_os.environ.get("ANGLE_SQ_GP", "1"))  # planes squared on GpSimd
SQ_VE = 6 - SQ_SC - SQ_GP
PD_ON_GP = _os.environ.get("ANGLE_PD_GP", "0") == "1"
NUM_DEV = int(_os.environ.get("ANGLE_NUM_DEV", "1"))
assert sum(TILE_SIZES) == FREE and 0 <= SQ_VE <= 6

_BUILD_CACHE = {}


def _build_nc():
    key = (TILE_SIZES, SQ_SC, SQ_GP, PD_ON_GP, NUM_DEV)
    if key in _BUILD_CACHE:
        return _BUILD_CACHE[key]

    from concourse import bacc, mybir

    AF = mybir.ActivationFunctionType
    OP = mybir.AluOpType
    f32 = mybir.dt.float32
    bf16 = mybir.dt.bfloat16

    sizes = list(TILE_SIZES)
    T = len(sizes)
    NB = min(N_INBUF, T)
    NQ = 4
    Fmax = max(sizes)
    offs = [0]
    for s in sizes:
        offs.append(offs[-1] + s)
    tot = {}
    slot_tot = [0] * NQ
    for i in range(T):
        slot_tot[i % NQ] += 16
        tot[i] = slot_tot[i % NQ]

    nc = bacc.Bacc(
        "TRN2", target_bir_lowering=False, debug=False, num_devices=NUM_DEV
    )
    x = nc.dram_tensor("x", [6 * P * FREE], bf16, kind="ExternalInput")
    out = nc.dram_tensor("out", [P, 16], f32, kind="ExternalOutput")
    xf = x.ap()

    def sb(name, shape, dtype):
        return nc.alloc_sbuf_tensor(name, list(shape), dtype).ap()

    inbuf = [sb(f"inb{b}", [P, 6 * Fmax], bf16) for b in range(NB)]
    sqb = [sb(f"sqb{b}", [P, 6 * Fmax], bf16) for b in range(2)]
    m = sb("m", [P, 3 * Fmax], bf16)
    dxy = sb("dxy", [P, Fmax], bf16)
    dotb = [sb(f"dot{b}", [P, Fmax], bf16) for b in range(2)]
    d2b = [sb(f"d2{b}", [P, Fmax], bf16) for b in range(2)]
    pair = sb("pair", [P, 2 * Fmax], bf16)
    oottb = [sb(f"oott{b}", [P, 2 * Fmax], bf16) for b in range(2)]
    prodb = [sb(f"prod{b}", [P, Fmax], bf16) for b in range(2)]
    pdb = [sb(f"pd{b}", [P, Fmax], bf16) for b in range(2)]
    rrb = [sb(f"rr{b}", [P, Fmax], bf16) for b in range(2)]
    h_all = sb("h_all", [P, FREE], bf16)
    t_scr = sb("t_scr", [P, FREE], bf16)
    asum = sb("asum", [P, 16], f32)
    warm = sb("warm", [P, 1], bf16)
    bias0 = sb("bias0", [P, 1], f32)

    S_dmaq = [nc.alloc_semaphore(f"s_dma{q}") for q in range(NQ)]
    S_dmo = nc.alloc_semaphore("s_dmo")
    S_bias = nc.alloc_semaphore("s_bias")
    S_sqsc = nc.alloc_semaphore("s_sqsc")  # 1/tile: Sc squares done
    S_sqgp = nc.alloc_semaphore("s_sqgp")  # 1/tile: Gp squares done
    S_prod = nc.alloc_semaphore("s_prod")  # 1/tile: VE prod+d2 done
    S_pd = nc.alloc_semaphore("s_pd")  # 1/tile: pd written
    S_rr = nc.alloc_semaphore("s_rr")  # 1/tile: rr written
    S_veg = nc.alloc_semaphore("s_veg")  # 1/tile: h written
    S_fin = nc.alloc_semaphore("s_fin")

    def dma_wait(eng, i):
        eng.wait_ge(S_dmaq[i % NQ], tot[i])

    with nc.Block(no_gpsimd_drain=True) as block:

        @block.sync
        def _(sync):
            for i in range(T):
                if i >= NB:
                    # inbuf[i%NB] free when all 3 square-readers + VE m
                    # of tile i-NB are done (S_prod covers VE's reads)
                    sync.wait_ge(S_prod, i - NB + 1)
                    sync.wait_ge(S_sqsc, i - NB + 1)
                    sync.wait_ge(S_sqgp, i - NB + 1)
                tile = xf[6 * P * offs[i] : 6 * P * offs[i + 1]].rearrange(
                    "(p f) -> p f", p=P
                )
                sync.dma_start(
                    out=inbuf[i % NB][:, : 6 * sizes[i]], in_=tile
                ).then_inc(S_dmaq[i % NQ], 16)
            sync.wait_ge(S_fin, 1)
            sync.dma_start(out=out.ap()[:, :], in_=asum[:, :]).then_inc(
                S_dmo, 16
            )
            sync.wait_ge(S_dmo, 16)

        @block.gpsimd
        def _(gpsimd):
            if not SQ_GP:
                gpsimd.memset(dxy[:, :1], 0.0).then_inc(S_sqgp, T)
            for i in range(T):
                h = i % 2
                F = sizes[i]
                if SQ_GP:
                    lo = 6 - SQ_GP  # planes [lo:6) on GpSimd
                    dma_wait(gpsimd, i)
                    if i >= 2:
                        gpsimd.wait_ge(S_prod, i - 1)  # sqb[h] free
                    gpsimd.tensor_tensor(
                        sqb[h][:, lo * F : 6 * F],
                        inbuf[i % NB][:, lo * F : 6 * F],
                        inbuf[i % NB][:, lo * F : 6 * F],
                        OP.mult,
                    ).then_inc(S_sqgp)
                if PD_ON_GP:
                    gpsimd.wait_ge(S_prod, i + 1)
                    if i >= 2:
                        gpsimd.wait_ge(S_rr, i - 1)  # pdb[h] free
                    gpsimd.tensor_tensor(
                        pdb[h][:, :F], prodb[h][:, :F], d2b[h][:, :F],
                        OP.subtract,
                    ).then_inc(S_pd)

        @block.vector
        def _(vector):
            vector.memset(bias0[:], 0.0).then_inc(S_bias)
            vector.memset(asum[:, :], 0.0).then_inc(S_bias)
            for i in range(T + 1):
                h = i % 2
                hp = (i - 1) % 2
                if i < T:
                    F = sizes[i]
                    inb = inbuf[i % NB]
                    dma_wait(vector, i)
                    vector.tensor_tensor(
                        m[:, : 3 * F], inb[:, : 3 * F], inb[:, 3 * F : 6 * F],
                        OP.mult,
                    )
                    if SQ_VE:
                        # independent of m: buries the m->dxy RAW bubble
                        if i >= 2:
                            vector.wait_ge(S_prod, i - 1)  # sqb[h] free
                        lo = SQ_SC  # planes [SQ_SC, SQ_SC+SQ_VE) on VE
                        vector.tensor_tensor(
                            sqb[h][:, lo * F : (lo + SQ_VE) * F],
                            inb[:, lo * F : (lo + SQ_VE) * F],
                            inb[:, lo * F : (lo + SQ_VE) * F],
                            OP.mult,
                        )
                    vector.tensor_tensor(
                        dxy[:, :F], m[:, :F], m[:, F : 2 * F], OP.add
                    )
                    vector.tensor_tensor(
                        dotb[h][:, :F], dxy[:, :F], m[:, 2 * F : 3 * F],
                        OP.add,
                    )
                    if PD_ON_GP and i >= 2:
                        # d2b/prodb[h] free only once Gp's pd of tile i-2
                        # has read them
                        vector.wait_ge(S_pd, i - 1)
                    vector.tensor_tensor(
                        d2b[h][:, :F], dotb[h][:, :F], dotb[h][:, :F],
                        OP.mult,
                    )
                    if i >= 1:
                        # h of tile i-1 here: rr is usually ready by now,
                        # and it fills the wait for Sc/Gp squares of tile i
                        Fp = sizes[i - 1]
                        vector.wait_ge(S_rr, i)
                        vector.tensor_tensor(
                            h_all[:, offs[i - 1] : offs[i]],
                            dotb[hp][:, :Fp], rrb[hp][:, :Fp], OP.mult,
                        ).then_inc(S_veg)
                    if not SQ_VE and i >= 2:
                        vector.wait_ge(S_prod, i - 1)  # sqb[h] free
                    vector.wait_ge(S_sqsc, i + 1)
                    vector.wait_ge(S_sqgp, i + 1)
                    sq6 = sqb[h][:, : 6 * F].rearrange("p (j f) -> p j f", j=6)
                    pr = pair[:, : 2 * F].rearrange("p (j f) -> p j f", j=2)
                    ot = oottb[h][:, : 2 * F].rearrange(
                        "p (j f) -> p j f", j=2
                    )
                    vector.tensor_tensor(
                        pr[:], sq6[:, 0:5:3, :], sq6[:, 1:6:3, :], OP.add
                    )
                    vector.tensor_tensor(
                        ot[:], pr[:], sq6[:, 2:6:3, :], OP.add
                    )
                    vector.tensor_tensor(
                        prodb[h][:, :F], ot[:, 0, :], ot[:, 1, :], OP.mult
                    ).then_inc(S_prod)
                    if not PD_ON_GP:
                        vector.tensor_tensor(
                            pdb[h][:, :F], prodb[h][:, :F], d2b[h][:, :F],
                            OP.subtract,
                        ).then_inc(S_pd)
                else:
                    # epilogue: h of the last tile
                    F = sizes[i - 1]
                    vector.wait_ge(S_rr, i)
                    vector.tensor_tensor(
                        h_all[:, offs[i - 1] : offs[i]],
                        dotb[hp][:, :F], rrb[hp][:, :F], OP.mult,
                    ).then_inc(S_veg)

        @block.scalar
        def _(scalar):
            # first activation pins the absrsqrt table set
            scalar.activation(
                warm[:], warm[:], AF.Abs_reciprocal_sqrt, bias=warm[:],
                scale=0.0,
            )
            scalar.wait_ge(S_bias, 2)
            if not SQ_SC:
                scalar.activation(
                    warm[:], warm[:], AF.Copy, bias=0.0, scale=0.0
                ).then_inc(S_sqsc, T)
            for i in range(T):
                h = i % 2
                F = sizes[i]
                if SQ_SC:
                    dma_wait(scalar, i)
                    if i >= 2:
                        scalar.wait_ge(S_prod, i - 1)  # sqb[h] free
                    scalar.activation(
                        sqb[h][:, : SQ_SC * F],
                        inbuf[i % NB][:, : SQ_SC * F],
                        AF.Square, bias=bias0[:],
                    ).then_inc(S_sqsc)
                scalar.wait_ge(S_pd, i + 1)
                if i >= 2:
                    scalar.wait_ge(S_veg, i - 1)  # rrb[h] free
                scalar.activation(
                    rrb[h][:, :F], pdb[h][:, :F], AF.Abs_reciprocal_sqrt,
                    bias=bias0[:],
                ).then_inc(S_rr)
            # table switch to the arctan set while VE finishes h tiles
            scalar.activation(
                warm[:], warm[:], AF.Arctan, bias=bias0[:], scale=0.0
            )
            if T > 1:
                scalar.wait_ge(S_veg, T - 1)
                scalar.activation(
                    t_scr[:, : offs[T - 1]], h_all[:, : offs[T - 1]],
                    AF.Arctan, bias=bias0[:], accum_out=asum[:, 0:1],
                )
            scalar.wait_ge(S_veg, T)
            scalar.activation(
                t_scr[:, offs[T - 1] :], h_all[:, offs[T - 1] :],
                AF.Arctan, bias=bias0[:], accum_out=asum[:, 1:2],
            )
            # trailing in-order op so the out-DMA can't beat READ_ACCUMULATOR
            scalar.activation(
                warm[:], warm[:], AF.Copy, bias=0.0, scale=0.0
            ).then_inc(S_fin)

    nc.compile()
    _BUILD_CACHE[key] = nc
    return nc


def _shard_inputs(outputs, targets):
    import ml_dtypes

    bf = ml_dtypes.bfloat16
    o = np.asarray(outputs, dtype=np.float32).reshape(-1, 3).astype(bf)
    t = np.asarray(targets, dtype=np.float32).reshape(-1, 3).astype(bf)
    in_maps = []
    for cidx in range(N_CORES):
        lo, hi = cidx * PER_CORE, (cidx + 1) * PER_CORE
        oc = o[lo:hi]
        tc_ = t[lo:hi]
        planes = np.empty((6, P, FREE), dtype=bf)
        for k in range(3):
            planes[k] = oc[:, k].reshape(P, FREE)
            planes[3 + k] = tc_[:, k].reshape(P, FREE)
        blocks = []
        off = 0
        for F in TILE_SIZES:
            blk = planes[:, :, off : off + F]  # [6, P, F]
            blocks.append(
                np.ascontiguousarray(blk.transpose(1, 0, 2)).reshape(-1)
            )
            off += F
        in_maps.append({"x": np.concatenate(blocks)})
    return in_maps


LAST_RESULT = None


def kernel(outputs, targets):
    global LAST_RESULT
    import os

    from concourse.bass_utils import run_bass_kernel_spmd

    nc = _build_nc()
    in_maps = _shard_inputs(outputs, targets)
    trace = bool(os.environ.get("ANGLE_KERNEL_TRACE"))
    res = run_bass_kernel_spmd(
        nc, in_maps, core_ids=list(range(N_CORES)), trace=trace
    )
    LAST_RESULT = res
    total = 0.0
    for rmap in res.results:
        total += np.asarray(rmap["out"], dtype=np.float64)[:, 0:2].sum()
    mean = np.pi / 2.0 - total / R_TOTAL
    return np.float32(mean)


# revision 13
# speedup vs baseline: 1.1872x; 1.1872x over previous
"""AngleLoss distributed Trainium2 kernel, v2.

mean(arccos(dot(o,t)/(|o||t|))) over 2,097,152 rows of 3-vectors,
data-parallel over 8 NeuronCores. Host pre-rounds inputs to bf16
(halves HBM traffic; rel-err budget 2e-2 is ~100x above the cost).

Math per row (one LUT chain, no explicit cos):
    dot = sum o*t ; prod = (sum o^2)(sum t^2)
    h   = dot * absrsqrt(|prod - dot^2|)   # = cot(theta)
    theta = pi/2 - arctan(h)               # arctan covers +-inf -> +-pi/2
Eliminates v1's r1/c/c2/nump stages (saves ~4C VE elems + 1 ScalarE LUT).
Device accumulates sum(arctan(h)); host computes pi/2 - total/R.

Per-tile engine split (tunable): squares of the 6 planes are divided
between ScalarE (Square activation), GpSimd (tensor_tensor), and VE;
pd = prod - dot^2 optionally on GpSimd. VE does m/dot/pairs/prod/h.

Layout: per core, tile-major planar: tile i = [128 part x 6*F_i bf16]
with each partition's 6*F_i values contiguous (planes o0,o1,o2,t0,t1,t2).
"""

import sys

import numpy as np

if "/opt/trn_rl_repo" not in sys.path:
    sys.path.insert(0, "/opt/trn_rl_repo")

N_CORES = 8
R_TOTAL = 256 * 8192  # 2097152 rows
PER_CORE = R_TOTAL // N_CORES  # 262144
P = 128
FREE = PER_CORE // P  # 2048

import os as _os

_ts = _os.environ.get("ANGLE_TILE_SIZES")
TILE_SIZES = tuple(int(v) for v in _ts.split(",")) if _ts else (256, 576, 704, 512)
N_INBUF = len(TILE_SIZES) + 1  # all tiles resident + 1
SQ_SC = int(_os.environ.get("ANGLE_SQ_SC", "3"))  # planes squared on ScalarE
SQ_GP = int(_os.environ.get("ANGLE_SQ_GP", "1"))  # planes squared on GpSimd
SQ_VE = 6 - SQ_SC - SQ_GP
PD_ON_GP = _os.environ.get("ANGLE_PD_GP", "0") == "1"
NUM_DEV = int(_os.environ.get("ANGLE_NUM_DEV", "1"))
assert sum(TILE_SIZES) == FREE and 0 <= SQ_VE <= 6

_BUILD_CACHE = {}


def _build_nc():
    key = (TILE_SIZES, SQ_SC, SQ_GP, PD_ON_GP, NUM_DEV)
    if key in _BUILD_CACHE:
        return _BUILD_CACHE[key]

    from concourse import bacc, mybir

    AF = mybir.ActivationFunctionType
    OP = mybir.AluOpType
    f32 = mybir.dt.float32
    bf16 = mybir.dt.bfloat16

    sizes = list(TILE_SIZES)
    T = len(sizes)
    NB = min(N_INBUF, T)
    NQ = 4
    Fmax = max(sizes)
    offs = [0]
    for s in sizes:
        offs.append(offs[-1] + s)
    tot = {}
    slot_tot = [0] * NQ
    for i in range(T):
        slot_tot[i % NQ] += 16
        tot[i] = slot_tot[i % NQ]

    nc = bacc.Bacc(
        "TRN2", target_bir_lowering=False, debug=False, num_devices=NUM_DEV
    )
    x = nc.dram_tensor("x", [6 * P * FREE], bf16, kind="ExternalInput")
    out = nc.dram_tensor("out", [P, 16], f32, kind="ExternalOutput")
    xf = x.ap()

    def sb(name, shape, dtype):
        return nc.alloc_sbuf_tensor(name, list(shape), dtype).ap()

    inbuf = [sb(f"inb{b}", [P, 6 * Fmax], bf16) for b in range(NB)]
    sqb = [sb(f"sqb{b}", [P, 6 * Fmax], bf16) for b in range(2)]
    m = sb("m", [P, 3 * Fmax], bf16)
    dxy = sb("dxy", [P, Fmax], bf16)
    pair = sb("pair", [P, 2 * Fmax], bf16)
    oottb = [sb(f"oott{b}", [P, 3 * Fmax], bf16) for b in range(2)]
    qb = [sb(f"q{b}", [P, 2 * Fmax], bf16) for b in range(2)]
    pdb = [sb(f"pd{b}", [P, Fmax], bf16) for b in range(2)]
    rrb = [sb(f"rr{b}", [P, Fmax], bf16) for b in range(2)]
    h_all = sb("h_all", [P, FREE], bf16)
    t_scr = sb("t_scr", [P, FREE], bf16)
    asum = sb("asum", [P, 16], f32)
    warm = sb("warm", [P, 1], bf16)
    bias0 = sb("bias0", [P, 1], f32)

    S_dmaq = [nc.alloc_semaphore(f"s_dma{q}") for q in range(NQ)]
    S_dmo = nc.alloc_semaphore("s_dmo")
    S_bias = nc.alloc_semaphore("s_bias")
    S_sqsc = nc.alloc_semaphore("s_sqsc")  # 1/tile: Sc squares done
    S_sqgp = nc.alloc_semaphore("s_sqgp")  # 1/tile: Gp squares done
    S_prod = nc.alloc_semaphore("s_prod")  # 1/tile: VE prod+d2 done
    S_pd = nc.alloc_semaphore("s_pd")  # 1/tile: pd written
    S_rr = nc.alloc_semaphore("s_rr")  # 1/tile: rr written
    S_veg = nc.alloc_semaphore("s_veg")  # 1/tile: h written
    S_fin = nc.alloc_semaphore("s_fin")

    def dma_wait(eng, i):
        eng.wait_ge(S_dmaq[i % NQ], tot[i])

    with nc.Block(no_gpsimd_drain=True) as block:

        @block.sync
        def _(sync):
            for i in range(T):
                if i >= 3:
                    # pace the stream: at most ~3 tiles in flight so DMA
                    # SBUF-port traffic doesn't pile onto early compute
                    sync.wait_ge(S_prod, i - 2)
                if i >= NB:
                    sync.wait_ge(S_prod, i - NB + 1)
                    sync.wait_ge(S_sqsc, i - NB + 1)
                    sync.wait_ge(S_sqgp, i - NB + 1)
                tile = xf[6 * P * offs[i] : 6 * P * offs[i + 1]].rearrange(
                    "(p f) -> p f", p=P
                )
                sync.dma_start(
                    out=inbuf[i % NB][:, : 6 * sizes[i]], in_=tile
                ).then_inc(S_dmaq[i % NQ], 16)
            sync.wait_ge(S_fin, 1)
            sync.dma_start(out=out.ap()[:, :], in_=asum[:, :]).then_inc(
                S_dmo, 16
            )
            sync.wait_ge(S_dmo, 16)

        @block.gpsimd
        def _(gpsimd):
            if not SQ_GP:
                gpsimd.memset(dxy[:, :1], 0.0).then_inc(S_sqgp, T)
            for i in range(T):
                h = i % 2
                F = sizes[i]
                if SQ_GP:
                    lo = 6 - SQ_GP  # planes [lo:6) on GpSimd
                    dma_wait(gpsimd, i)
                    if i >= 2:
                        gpsimd.wait_ge(S_prod, i - 1)  # sqb[h] free
                    gpsimd.tensor_tensor(
                        sqb[h][:, lo * F : 6 * F],
                        inbuf[i % NB][:, lo * F : 6 * F],
                        inbuf[i % NB][:, lo * F : 6 * F],
                        OP.mult,
                    ).then_inc(S_sqgp)
                if PD_ON_GP:
                    gpsimd.wait_ge(S_prod, i + 1)
                    if i >= 2:
                        gpsimd.wait_ge(S_rr, i - 1)  # pdb[h] free
                    gpsimd.tensor_tensor(
                        pdb[h][:, :F], qb[h][:, :F], qb[h][:, F : 2 * F],
                        OP.subtract,
                    ).then_inc(S_pd)

        @block.vector
        def _(vector):
            vector.memset(bias0[:], 0.0).then_inc(S_bias)
            vector.memset(asum[:, :], 0.0).then_inc(S_bias)
            for i in range(T + 1):
                h = i % 2
                hp = (i - 1) % 2
                if i < T:
                    F = sizes[i]
                    inb = inbuf[i % NB]
                    dma_wait(vector, i)
                    vector.tensor_tensor(
                        m[:, : 3 * F], inb[:, : 3 * F], inb[:, 3 * F : 6 * F],
                        OP.mult,
                    )
                    if SQ_VE:
                        # independent of m: buries the m->dxy RAW bubble
                        if i >= 2:
                            vector.wait_ge(S_prod, i - 1)  # sqb[h] free
                        lo = SQ_SC  # planes [SQ_SC, SQ_SC+SQ_VE) on VE
                        vector.tensor_tensor(
                            sqb[h][:, lo * F : (lo + SQ_VE) * F],
                            inb[:, lo * F : (lo + SQ_VE) * F],
                            inb[:, lo * F : (lo + SQ_VE) * F],
                            OP.mult,
                        )
                    vector.tensor_tensor(
                        dxy[:, :F], m[:, :F], m[:, F : 2 * F], OP.add
                    )
                    # dot lands in the third lane of oott: {oo, tt, dot}
                    vector.tensor_tensor(
                        oottb[h][:, 2 * F : 3 * F], dxy[:, :F],
                        m[:, 2 * F : 3 * F], OP.add,
                    )
                    if i >= 1:
                        # h of tile i-1 here: rr is usually ready by now,
                        # and it fills the wait for Sc/Gp squares of tile i
                        Fp = sizes[i - 1]
                        vector.wait_ge(S_rr, i)
                        vector.tensor_tensor(
                            h_all[:, offs[i - 1] : offs[i]],
                            oottb[hp][:, 2 * Fp : 3 * Fp], rrb[hp][:, :Fp],
                            OP.mult,
                        ).then_inc(S_veg)
                    if i >= 2:
                        vector.wait_ge(S_prod, i - 1)  # sqb/oott[h] free
                    vector.wait_ge(S_sqsc, i + 1)
                    vector.wait_ge(S_sqgp, i + 1)
                    sq6 = sqb[h][:, : 6 * F].rearrange("p (j f) -> p j f", j=6)
                    pr = pair[:, : 2 * F].rearrange("p (j f) -> p j f", j=2)
                    o3 = oottb[h][:, : 3 * F].rearrange(
                        "p (j f) -> p j f", j=3
                    )
                    vector.tensor_tensor(
                        pr[:], sq6[:, 0:5:3, :], sq6[:, 1:6:3, :], OP.add
                    )
                    vector.tensor_tensor(
                        o3[:, 0:2, :], pr[:], sq6[:, 2:6:3, :], OP.add
                    )
                    if PD_ON_GP and i >= 2:
                        vector.wait_ge(S_pd, i - 1)  # qb[h] free
                    # q = {oo,dot} * {tt,dot} = {oo*tt, dot^2} in one op
                    vector.tensor_tensor(
                        qb[h][:, : 2 * F], o3[:, 0:3:2, :], o3[:, 1:3, :],
                        OP.mult,
                    ).then_inc(S_prod)
                    if not PD_ON_GP:
                        vector.tensor_tensor(
                            pdb[h][:, :F], qb[h][:, :F], qb[h][:, F : 2 * F],
                            OP.subtract,
                        ).then_inc(S_pd)
                else:
                    # epilogue: h of the last tile
                    F = sizes[i - 1]
                    vector.wait_ge(S_rr, i)
                    vector.tensor_tensor(
                        h_all[:, offs[i - 1] : offs[i]],
                        oottb[hp][:, 2 * F : 3 * F], rrb[hp][:, :F], OP.mult,
                    ).then_inc(S_veg)

        @block.scalar
        def _(scalar):
            # first activation pins the absrsqrt table set
            scalar.activation(
                warm[:], warm[:], AF.Abs_reciprocal_sqrt, bias=warm[:],
                scale=0.0,
            )
            scalar.wait_ge(S_bias, 2)
            if not SQ_SC:
                scalar.activation(
                    warm[:], warm[:], AF.Copy, bias=0.0, scale=0.0
                ).then_inc(S_sqsc, T)
            for i in range(T):
                h = i % 2
                F = sizes[i]
                if SQ_SC:
                    dma_wait(scalar, i)
                    if i >= 2:
                        scalar.wait_ge(S_prod, i - 1)  # sqb[h] free
                    scalar.activation(
                        sqb[h][:, : SQ_SC * F],
                        inbuf[i % NB][:, : SQ_SC * F],
                        AF.Square, bias=bias0[:],
                    ).then_inc(S_sqsc)
                scalar.wait_ge(S_pd, i + 1)
                if i >= 2:
                    scalar.wait_ge(S_veg, i - 1)  # rrb[h] free
                scalar.activation(
                    rrb[h][:, :F], pdb[h][:, :F], AF.Abs_reciprocal_sqrt,
                    bias=bias0[:],
                ).then_inc(S_rr)
            # table switch to the arctan set while VE finishes h tiles
            scalar.activation(
                warm[:], warm[:], AF.Arctan, bias=bias0[:], scale=0.0
            )
            if T > 1:
                scalar.wait_ge(S_veg, T - 1)
                scalar.activation(
                    t_scr[:, : offs[T - 1]], h_all[:, : offs[T - 1]],
                    AF.Arctan, bias=bias0[:], accum_out=asum[:, 0:1],
                )
            scalar.wait_ge(S_veg, T)
            scalar.activation(
                t_scr[:, offs[T - 1] :], h_all[:, offs[T - 1] :],
                AF.Arctan, bias=bias0[:], accum_out=asum[:, 1:2],
            )
            # trailing in-order op so the out-DMA can't beat READ_ACCUMULATOR
            scalar.activation(
                warm[:], warm[:], AF.Copy, bias=0.0, scale=0.0
            ).then_inc(S_fin)

    nc.compile()
    _BUILD_CACHE[key] = nc
    return nc


def _shard_inputs(outputs, targets):
    import ml_dtypes

    bf = ml_dtypes.bfloat16
    o = np.asarray(outputs, dtype=np.float32).reshape(-1, 3).astype(bf)
    t = np.asarray(targets, dtype=np.float32).reshape(-1, 3).astype(bf)
    in_maps = []
    for cidx in range(N_CORES):
        lo, hi = cidx * PER_CORE, (cidx + 1) * PER_CORE
        oc = o[lo:hi]
        tc_ = t[lo:hi]
        planes = np.empty((6, P, FREE), dtype=bf)
        for k in range(3):
            planes[k] = oc[:, k].reshape(P, FREE)
            planes[3 + k] = tc_[:, k].reshape(P, FREE)
        blocks = []
        off = 0
        for F in TILE_SIZES:
            blk = planes[:, :, off : off + F]  # [6, P, F]
            blocks.append(
                np.ascontiguousarray(blk.transpose(1, 0, 2)).reshape(-1)
            )
            off += F
        in_maps.append({"x": np.concatenate(blocks)})
    return in_maps


LAST_RESULT = None


def kernel(outputs, targets):
    global LAST_RESULT
    import os

    from concourse.bass_utils import run_bass_kernel_spmd

    nc = _build_nc()
    in_maps = _shard_inputs(outputs, targets)
    trace = bool(os.environ.get("ANGLE_KERNEL_TRACE"))
    res = run_bass_kernel_spmd(
        nc, in_maps, core_ids=list(range(N_CORES)), trace=trace
    )
    LAST_RESULT = res
    total = 0.0
    for rmap in res.results:
        total += np.asarray(rmap["out"], dtype=np.float64)[:, 0:2].sum()
    mean = np.pi / 2.0 - total / R_TOTAL
    return np.float32(mean)
